# revision 3
# baseline (speedup 1.0000x reference)
"""Trainium2 Bass kernel for nn_FCLSTM: embedding -> custom LSTM-ish recurrence
-> select -> linear -> log_softmax.

v3: batch-sharded design. 8 cores x 8 batch rows each, weights replicated,
ZERO collectives. Per step, the two gate matmuls x two n-halves run as a
4-way column-tiled pack on the PE (M=8 occupies only 8 of each 32-col group,
so 4 matmuls stream concurrently through disjoint col groups), halving
weight-stream time vs a 2-way pack.

Layout invariants (walrus enforces tensor_tensor same-start-partition):
  psum bank A: tanh-gate (Wh) quarters: n-half0 at rows 0:8, n-half1 at 32:40
  psum bank B: sigm-gate (Wf) quarters: n-half0 at rows 64:72, n-half1 at 96:104
  ACT tanh bank A in place; ACT sigmoid bank B shifted -64 -> both tiles have
  half0 at rows 0:8 and half1 at rows 32:40; inp staged at base 0 (half0) and
  base 32 (half1) so every DVE op is base-aligned.

Embedding lookup happens HOST-side (numpy fancy-index, same spirit as the
host-side weight transposes); the e @ Wi.T matmul runs on-device in 16-step
chunks interleaved with the recurrence.

Self-contained: hardcodes shapes. kernel(**inputs) takes full numpy inputs,
returns [64, 2] fp32 log-probs.
"""
import os
import numpy as np

import concourse.bacc as bacc
import concourse.bass as bass
import concourse.mybir as mybir
from concourse import library_config  # noqa: F401
from concourse.tile import TileContext
from concourse.bass_utils import run_bass_kernel_spmd

VOCAB, EMBED, HIDDEN, NCLS = 32000, 512, 1024, 2
B, S = 64, 512
NCORES = 8
BL = B // NCORES               # 8 local batch rows per core
NKC = HIDDEN // 128            # 8 hidden contraction chunks
NEC = EMBED // 128             # 4 embed contraction chunks
CHS = 16                       # steps per inp chunk (128 tokens)
F16 = mybir.dt.float16
F32 = mybir.dt.float32
I32 = mybir.dt.int32
Tanh = mybir.ActivationFunctionType.Tanh
Sigmoid = mybir.ActivationFunctionType.Sigmoid
Relu = mybir.ActivationFunctionType.Relu

_CACHE = {}


def _build(steps=S):
    nch = (steps + CHS - 1) // CHS
    nc = bacc.Bacc("TRN2", target_bir_lowering=False, debug=False, num_devices=NCORES)

    # ---------- inputs ----------
    egt = nc.dram_tensor("egt", [128, nch * NEC * 128], F16, kind="ExternalInput")
    wi = nc.dram_tensor("wi", [EMBED, HIDDEN], F16, kind="ExternalInput")
    birep = nc.dram_tensor("birep", [128, HIDDEN], F32, kind="ExternalInput")
    wf = nc.dram_tensor("wf", [HIDDEN, HIDDEN], F16, kind="ExternalInput")
    wh = nc.dram_tensor("wh", [HIDDEN, HIDDEN], F16, kind="ExternalInput")
    b4 = nc.dram_tensor("b4", [4, 512], F16, kind="ExternalInput")  # bh0,bh1,bf0,bf1
    wo = nc.dram_tensor("wo", [HIDDEN, HIDDEN], F16, kind="ExternalInput")
    bo_r = nc.dram_tensor("bo_r", [1, HIDDEN], F16, kind="ExternalInput")
    wlin = nc.dram_tensor("wlin", [HIDDEN, NCLS], F16, kind="ExternalInput")
    sel_d = nc.dram_tensor("sel_d", [4, 128], F16, kind="ExternalInput")
    id8_d = nc.dram_tensor("id8_d", [40, 8], F16, kind="ExternalInput")
    ones8_d = nc.dram_tensor("ones8_d", [1, 8], F16, kind="ExternalInput")
    selidx = nc.dram_tensor("selidx", [128, 1], I32, kind="ExternalInput")
    out_ext = nc.dram_tensor("out", [BL, NCLS], F32, kind="ExternalOutput")

    with TileContext(nc) as tc:
        with (
            tc.tile_pool(name="dram", bufs=1, space="DRAM") as dram,
            tc.tile_pool(name="w", bufs=1) as wpool,
            tc.tile_pool(name="cst", bufs=1) as cst,
            tc.tile_pool(name="eg", bufs=3) as egpool,
            tc.tile_pool(name="ibuf", bufs=3) as ibpool,
            tc.tile_pool(name="i2", bufs=3) as i2pool,
            tc.tile_pool(name="tail", bufs=2) as tpool,
            tc.tile_pool(name="rec", bufs=2) as rec,
            tc.tile_pool(name="gp", bufs=2, space="PSUM") as gp,
            tc.tile_pool(name="ip", bufs=1, space="PSUM") as ip,
            tc.tile_pool(name="tp", bufs=2, space="PSUM") as tp,
            tc.tile_pool(name="dum", bufs=1, space="PSUM") as dum_p,
        ):
            ring = dram.tile([steps * BL, HIDDEN], F16)

            # ---------- constants / weights ----------
            sel = cst.tile([4, 128], F16, tag="sel")
            nc.sync.dma_start(out=sel[:], in_=sel_d[:, :])
            id8 = cst.tile([40, 8], F16, tag="id8")
            nc.sync.dma_start(out=id8[:], in_=id8_d[:, :])
            ones8 = cst.tile([1, 8], F16, tag="ones8")
            nc.sync.dma_start(out=ones8[:], in_=ones8_d[:, :])
            b4_sb = cst.tile([4, 512], F16, tag="b4")
            nc.sync.dma_start(out=b4_sb[:], in_=b4[:, :])
            birep_sb = cst.tile([128, HIDDEN], F32, tag="birep")
            nc.sync.dma_start(out=birep_sb[:], in_=birep[:, :])
            wi_sb = cst.tile([128, NEC * HIDDEN], F16, tag="wi")
            for e in range(NEC):
                nc.sync.dma_start(out=wi_sb[:, e * HIDDEN:(e + 1) * HIDDEN],
                                  in_=wi[e * 128:(e + 1) * 128, :])
            wf_sb = wpool.tile([128, NKC * HIDDEN], F16, tag="wf")
            wh_sb = wpool.tile([128, NKC * HIDDEN], F16, tag="wh")
            for k in range(NKC):
                nc.sync.dma_start(out=wf_sb[:, k * HIDDEN:(k + 1) * HIDDEN],
                                  in_=wf[k * 128:(k + 1) * 128, :])
                nc.sync.dma_start(out=wh_sb[:, k * HIDDEN:(k + 1) * HIDDEN],
                                  in_=wh[k * 128:(k + 1) * 128, :])

            # ---------- inp chunk pipeline (emitted interleaved below) ----------
            ibufs = {}

            inp_state = {}

            def emit_inp_sub(g, e):
                if e == 0:
                    eg = egpool.tile([128, NEC * 128], F16, tag="eg", name=f"eg{g}")
                    nc.sync.dma_start(out=eg[:],
                                      in_=egt[:, g * NEC * 128:(g + 1) * NEC * 128])
                    piA = ip.tile([128, 512], F32, tag="piA")
                    piB = ip.tile([128, 512], F32, tag="piB")
                    inp_state[g] = (eg, piA, piB)
                eg, piA, piB = inp_state[g]
                nc.tensor.matmul(out=piA[:], lhsT=eg[:, e * 128:(e + 1) * 128],
                                 rhs=wi_sb[:, e * HIDDEN:e * HIDDEN + 512],
                                 start=(e == 0), stop=(e == NEC - 1))
                nc.tensor.matmul(out=piB[:], lhsT=eg[:, e * 128:(e + 1) * 128],
                                 rhs=wi_sb[:, e * HIDDEN + 512:(e + 1) * HIDDEN],
                                 start=(e == 0), stop=(e == NEC - 1))
                if e == NEC - 1:
                    ib = ibpool.tile([128, HIDDEN], F16, tag="ib", name=f"ib{g}")
                    nc.vector.tensor_add(out=piA[:], in0=piA[:],
                                         in1=birep_sb[:, 0:512])
                    nc.scalar.activation(ib[:, 0:512], piA[:], Relu)
                    nc.vector.tensor_add(out=piB[:], in0=piB[:],
                                         in1=birep_sb[:, 512:1024])
                    nc.scalar.activation(ib[:, 512:1024], piB[:], Relu)
                    ibufs[g] = ib
                    del inp_state[g]

            def emit_inp_chunk(g):
                for e in range(NEC):
                    emit_inp_sub(g, e)

            def stage_i2(t):
                g, tl = t // CHS, t % CHS
                ib = ibufs[g]
                i2 = i2pool.tile([8, 1024], F16, tag="i2", name="i2")
                nc.gpsimd.dma_start(out=i2[:], in_=ib[tl * 8:(tl + 1) * 8, :])
                i2b = i2pool.tile([40, 512], F16, tag="i2b", name="i2b")
                nc.gpsimd.dma_start(out=i2b[32:40, :],
                                    in_=ib[tl * 8:(tl + 1) * 8, 512:1024])
                return i2, i2b

            emit_inp_chunk(0)
            if nch > 1:
                emit_inp_chunk(1)
            i2_cur = stage_i2(0)

            # ---------- recurrence ----------
            hT = rec.tile([128, 64], F16, tag="hT", name="hT0")
            nc.vector.memset(hT[:], 0.0)

            for t in range(steps):
                g, tl = t // CHS, t % CHS
                if tl % 4 == 0 and g + 2 < nch:
                    emit_inp_sub(g + 2, tl // 4)
                i2, i2b = i2_cur
                if t + 1 < steps:
                    i2_nxt = stage_i2(t + 1)

                pg = gp.tile([128, 512], F32, tag="pg")
                nc.tensor.matmul(out=pg[0:128, :], lhsT=sel[:, :], rhs=b4_sb[:, :],
                                 start=True, stop=False, tile_position=(0, 0))
                for k in range(NKC):
                    lhs = hT[:, k * 8:(k + 1) * 8]
                    last = k == NKC - 1
                    woff = k * HIDDEN
                    nc.tensor.matmul(out=pg[0:8, :], lhsT=lhs,
                                     rhs=wh_sb[:, woff:woff + 512],
                                     start=False, stop=last, tile_position=(0, 0))
                    nc.tensor.matmul(out=pg[32:40, :], lhsT=lhs,
                                     rhs=wh_sb[:, woff + 512:woff + 1024],
                                     start=False, stop=last, tile_position=(0, 32))
                    nc.tensor.matmul(out=pg[64:72, :], lhsT=lhs,
                                     rhs=wf_sb[:, woff:woff + 512],
                                     start=False, stop=last, tile_position=(0, 64))
                    nc.tensor.matmul(out=pg[96:104, :], lhsT=lhs,
                                     rhs=wf_sb[:, woff + 512:woff + 1024],
                                     start=False, stop=last, tile_position=(0, 96))

                thT = tpool.tile([40, 512], F16, tag="thT", name="thT")
                nc.scalar.activation(thT[0:40, :], pg[0:40, :], Tanh)
                thS = tpool.tile([40, 512], F16, tag="thS", name="thS")
                nc.scalar.activation(thS[0:40, :], pg[64:104, :], Sigmoid)
                # HAM fillers: keep the PE clock-gate warm during the ACT/DVE
                # tail. Depend on thT/thS so they run inside the gap; results
                # are discarded.
                dum = dum_p.tile([128, 512], F32, tag="dum")
                nc.tensor.matmul(out=dum[0:8, :], lhsT=thT[0:8, 0:8],
                                 rhs=thT[0:8, :], start=True, stop=False)
                nc.tensor.matmul(out=dum[0:8, :], lhsT=thT[0:8, 8:16],
                                 rhs=thT[0:8, :], start=False, stop=False)
                nc.tensor.matmul(out=dum[0:8, :], lhsT=thS[0:8, 0:8],
                                 rhs=thS[0:8, :], start=False, stop=False)
                nc.tensor.matmul(out=dum[0:8, :], lhsT=thS[0:8, 8:16],
                                 rhs=thS[0:8, :], start=False, stop=True)

                hnp = tpool.tile([8, 512], F16, tag="hnp", name="hnp")
                hnp1 = tpool.tile([40, 512], F16, tag="hnp1", name="hnp1")
                nc.vector.tensor_mul(out=hnp[0:8, :], in0=thT[0:8, :],
                                     in1=i2[0:8, 0:512])
                nc.vector.tensor_add(out=hnp[0:8, :], in0=hnp[0:8, :],
                                     in1=thS[0:8, :])

                last_step = t == steps - 1
                if not last_step:
                    pt = tp.tile([128, 64], F16, tag="pt")
                    hTn = rec.tile([128, 64], F16, tag="hT", name="hTn")
                    for k in range(4):
                        nc.tensor.transpose(out=pt[:, k * 8:(k + 1) * 8],
                                            in_=hnp[0:8, k * 128:(k + 1) * 128],
                                            identity=id8[0:8, :])
                    nc.vector.tensor_copy(out=hTn[:, 0:32], in_=pt[:, 0:32])

                nc.vector.tensor_mul(out=hnp1[32:40, :], in0=thT[32:40, :],
                                     in1=i2b[32:40, :])
                nc.vector.tensor_add(out=hnp1[32:40, :], in0=hnp1[32:40, :],
                                     in1=thS[32:40, :])
                if not last_step:
                    for k in range(4):
                        nc.tensor.transpose(out=pt[:, 32 + k * 8:32 + (k + 1) * 8],
                                            in_=hnp1[32:40, k * 128:(k + 1) * 128],
                                            identity=id8[32:40, :],
                                            tile_position=(32, 0))
                    nc.vector.tensor_copy(out=hTn[:, 32:64], in_=pt[:, 32:64])
                    hT = hTn

                nc.scalar.dma_start(out=ring[t * 8:(t + 1) * 8, 0:512], in_=hnp[0:8, :])
                nc.scalar.dma_start(out=ring[t * 8:(t + 1) * 8, 512:1024],
                                    in_=hnp1[32:40, :])
                if t + 1 < steps:
                    i2_cur = i2_nxt

            # ---------- epilogue: select + linear + log_softmax ----------
            wo_sb = wpool.tile([128, NKC * HIDDEN], F16, tag="wo")
            for k in range(NKC):
                nc.sync.dma_start(out=wo_sb[:, k * HIDDEN:(k + 1) * HIDDEN],
                                  in_=wo[k * 128:(k + 1) * 128, :])
            bo_sb = cst.tile([1, HIDDEN], F16, tag="bo")
            nc.sync.dma_start(out=bo_sb[:], in_=bo_r[:, :])
            wl_sb = cst.tile([128, NKC * NCLS], F16, tag="wl")
            for k in range(NKC):
                nc.sync.dma_start(out=wl_sb[:, k * NCLS:(k + 1) * NCLS],
                                  in_=wlin[k * 128:(k + 1) * 128, :])
            six = cst.tile([128, 1], I32, tag="six")
            nc.sync.dma_start(out=six[:], in_=selidx[:, :])
            hsel = cst.tile([128, HIDDEN], F16, tag="hsel")
            nc.gpsimd.indirect_dma_start(
                out=hsel[:], out_offset=None,
                in_=ring[:, :],
                in_offset=bass.IndirectOffsetOnAxis(ap=six[:, :1], axis=0))
            pt2 = tp.tile([128, 64], F16, tag="pt")
            for k in range(NKC):
                nc.tensor.transpose(out=pt2[:, k * 8:(k + 1) * 8],
                                    in_=hsel[0:8, k * 128:(k + 1) * 128],
                                    identity=id8[0:8, :])
            hselT = cst.tile([128, 64], F16, tag="hselT")
            nc.vector.tensor_copy(out=hselT[:], in_=pt2[:])
            plA = gp.tile([128, 512], F32, tag="pg")
            plB = gp.tile([128, 512], F32, tag="pg")
            nc.tensor.matmul(out=plA[0:8, :], lhsT=ones8[:, :], rhs=bo_sb[:, 0:512],
                             start=True, stop=False)
            nc.tensor.matmul(out=plB[0:8, :], lhsT=ones8[:, :], rhs=bo_sb[:, 512:1024],
                             start=True, stop=False)
            for k in range(NKC):
                lhs = hselT[:, k * 8:(k + 1) * 8]
                last = k == NKC - 1
                nc.tensor.matmul(out=plA[0:8, :], lhsT=lhs,
                                 rhs=wo_sb[:, k * HIDDEN:k * HIDDEN + 512],
                                 start=False, stop=last)
                nc.tensor.matmul(out=plB[0:8, :], lhsT=lhs,
                                 rhs=wo_sb[:, k * HIDDEN + 512:(k + 1) * HIDDEN],
                                 start=False, stop=last)
            lin = cst.tile([8, HIDDEN], F16, tag="lin")
            nc.vector.tensor_copy(out=lin[:, 0:512], in_=plA[0:8, :])
            nc.vector.tensor_copy(out=lin[:, 512:1024], in_=plB[0:8, :])
            pt3 = tp.tile([128, 64], F16, tag="pt")
            for k in range(NKC):
                nc.tensor.transpose(out=pt3[:, k * 8:(k + 1) * 8],
                                    in_=lin[0:8, k * 128:(k + 1) * 128],
                                    identity=id8[0:8, :])
            linT = cst.tile([128, 64], F16, tag="linT")
            nc.vector.tensor_copy(out=linT[:], in_=pt3[:])
            pz = ip.tile([128, 512], F32, tag="piA")
            for k in range(NKC):
                nc.tensor.matmul(out=pz[0:8, 0:NCLS], lhsT=linT[:, k * 8:(k + 1) * 8],
                                 rhs=wl_sb[:, k * NCLS:(k + 1) * NCLS],
                                 start=(k == 0), stop=(k == NKC - 1))
            m = cst.tile([8, 1], F32, tag="m")
            nc.vector.tensor_reduce(out=m[:], in_=pz[0:8, 0:NCLS],
                                    axis=mybir.AxisListType.X, op=mybir.AluOpType.max)
            xm = cst.tile([8, NCLS], F32, tag="xm")
            nc.vector.tensor_scalar(out=xm[:], in0=pz[0:8, 0:NCLS], scalar1=m[:],
                                    scalar2=None, op0=mybir.AluOpType.subtract)
            esum = cst.tile([8, 1], F32, tag="esum")
            ex = cst.tile([8, NCLS], F32, tag="ex")
            nc.scalar.activation(ex[:], xm[:], mybir.ActivationFunctionType.Exp,
                                 accum_out=esum[:])
            lns = cst.tile([8, 1], F32, tag="lns")
            nc.scalar.activation(lns[:], esum[:], mybir.ActivationFunctionType.Ln)
            res = cst.tile([8, NCLS], F32, tag="res")
            nc.vector.tensor_scalar(out=res[:], in0=xm[:], scalar1=lns[:],
                                    scalar2=None, op0=mybir.AluOpType.subtract)
            nc.sync.dma_start(out=out_ext[:, :], in_=res[:])

    nc.compile()
    return nc


def _prep(x, lengths, emb, W_i, b_i, W_f, b_f, W_h, b_h, W_o, b_o, W_lin, b_lin,
          steps=S):
    f16 = np.float16
    f32 = np.float32
    nch = (steps + CHS - 1) // CHS

    wi_t = np.ascontiguousarray(W_i.astype(f32).T.astype(f16))       # [512, 1024]
    wf_t = np.ascontiguousarray(W_f.astype(f32).T.astype(f16))       # [1024, 1024]
    wh_t = np.ascontiguousarray(W_h.astype(f32).T.astype(f16))
    wo_t = np.ascontiguousarray(W_o.astype(f32).T.astype(f16))
    wl_t = np.ascontiguousarray(W_lin.astype(f32).T.astype(f16))     # [1024, 2]
    birep = np.ascontiguousarray(
        np.broadcast_to(b_i.astype(f32), (128, HIDDEN)))
    b4 = np.stack([b_h[0:512], b_h[512:1024],
                   b_f[0:512], b_f[512:1024]]).astype(f16)
    bo_r = b_o[None, :].astype(f16)

    sel_np = np.zeros((4, 128), f16)
    for q in range(4):
        sel_np[q, q * 32:q * 32 + 8] = 1.0
    id8_np = np.zeros((40, 8), f16)
    id8_np[0:8, :] = np.eye(8, dtype=f16)
    id8_np[32:40, :] = np.eye(8, dtype=f16)
    ones8 = np.ones((1, 8), f16)

    E16 = emb.astype(f16)
    Eg = E16[np.asarray(x)[:, :steps]]           # [B, steps, 512] host-side lookup

    maps = []
    for c in range(NCORES):
        Ec = Eg[c * BL:(c + 1) * BL]             # [8, steps, 512]
        if steps % CHS != 0:
            pad = nch * CHS - steps
            Ec = np.concatenate(
                [Ec, np.zeros((BL, pad, EMBED), f16)], axis=1)
        arr = Ec.transpose(1, 0, 2)              # [steps_p, 8, 512]
        arr = arr.reshape(nch, CHS * BL, NEC, 128)   # [g, tok, e, p]
        egt = np.ascontiguousarray(
            arr.transpose(3, 0, 2, 1).reshape(128, nch * NEC * 128))
        lloc = lengths[c * BL:(c + 1) * BL].astype(np.int64)
        sel_rows = ((lloc - 1) * BL + np.arange(BL)).astype(np.int32)
        selpad = np.zeros((128, 1), np.int32)
        selpad[:BL, 0] = sel_rows
        maps.append({
            "egt": egt,
            "wi": wi_t,
            "birep": birep,
            "wf": wf_t,
            "wh": wh_t,
            "b4": b4,
            "wo": wo_t,
            "bo_r": bo_r,
            "wlin": wl_t,
            "sel_d": sel_np,
            "id8_d": id8_np,
            "ones8_d": ones8,
            "selidx": selpad,
        })
    return maps


def _run(inputs, steps=S, trace=False):
    key = steps
    if key not in _CACHE:
        _CACHE[key] = _build(steps)
    nc = _CACHE[key]
    maps = _prep(**inputs, steps=steps)
    res = run_bass_kernel_spmd(nc, maps, core_ids=list(range(NCORES)), trace=trace)
    return res


def assemble(res) -> np.ndarray:
    return np.concatenate([res.results[c]["out"] for c in range(NCORES)], axis=0)


def kernel(**inputs) -> np.ndarray:
    res = _run(inputs, steps=S, trace=False)
    return assemble(res)


if __name__ == "__main__":
    steps = int(os.environ.get("KSTEPS", "16"))
    rng = np.random.default_rng(0)
    x = rng.integers(0, VOCAB, size=(B, S)).astype(np.int64)
    lengths = rng.integers(1, steps + 1, size=(B,)).astype(np.int64)
    lengths[0] = steps
    s_e, s_h = 1 / np.sqrt(EMBED), 1 / np.sqrt(HIDDEN)
    ins = dict(
        x=x, lengths=lengths,
        emb=rng.normal(size=(VOCAB, EMBED)).astype(np.float32),
        W_i=rng.uniform(-s_e, s_e, (HIDDEN, EMBED)).astype(np.float32),
        b_i=rng.uniform(-s_e, s_e, (HIDDEN,)).astype(np.float32),
        W_f=rng.uniform(-s_h, s_h, (HIDDEN, HIDDEN)).astype(np.float32),
        b_f=rng.uniform(-s_h, s_h, (HIDDEN,)).astype(np.float32),
        W_h=rng.uniform(-s_h, s_h, (HIDDEN, HIDDEN)).astype(np.float32),
        b_h=rng.uniform(-s_h, s_h, (HIDDEN,)).astype(np.float32),
        W_o=rng.uniform(-s_h, s_h, (HIDDEN, HIDDEN)).astype(np.float32),
        b_o=rng.uniform(-s_h, s_h, (HIDDEN,)).astype(np.float32),
        W_lin=rng.uniform(-s_h, s_h, (NCLS, HIDDEN)).astype(np.float32),
        b_lin=np.zeros((NCLS,), np.float32),
    )

    def npref(steps):
        e = ins["emb"][x]
        h = np.zeros((B, HIDDEN), np.float32)
        outs = np.zeros((steps, B, HIDDEN), np.float32)
        for t in range(steps):
            et_ = e[:, t, :]
            inp = np.maximum(et_ @ ins["W_i"].T + ins["b_i"], 0)
            hf = 1 / (1 + np.exp(-(h @ ins["W_f"].T + ins["b_f"])))
            hh = np.tanh(h @ ins["W_h"].T + ins["b_h"])
            h = hf + hh * inp
            outs[t] = h
        li = outs[lengths - 1, np.arange(B)]
        lin = li @ ins["W_o"].T + ins["b_o"]
        lg = lin @ ins["W_lin"].T + ins["b_lin"]
        lg = lg - lg.max(1, keepdims=True)
        return lg - np.log(np.exp(lg).sum(1, keepdims=True))

    expected = npref(steps)
    res = _run(ins, steps=steps, trace=False)
    got = assemble(res)
    err = np.linalg.norm(got - expected) / np.linalg.norm(expected)
    print("expected[:3]:", expected[:3])
    print("got[:3]:", got[:3])
    print("rel_err:", err)
